# revision 14
# baseline (speedup 1.0000x reference)
"""Trainium2 Bass kernel for nn_Lowpass: EMA recurrence over time.

level_t = (1-s) * x_t + s * level_{t-1},  s = sigmoid(smoothing_var)

Strategy:
  - Data-parallel over batch: 16 batches -> 8 cores x 2 batches.
  - Time is processed in blocks of L=128 (the SBUF partition dim).
    Within a block, the whole recurrence is a lower-triangular matmul:
        y[j] = sum_{i<=j} A[j,i] x[i] + s^(j+1) * c        (c = carry)
        A[j,i] = (1-s) * s^(j-i)
    The carry term folds into the matmul as a row-0 fixup, since
    A @ (x + e_0 * (s/(1-s)) * c) = A @ x + p * c with p[j] = s^(j+1).
  - TensorE does the 128x128 prefix matmul (float32r: full-rate fp32),
    DVE does the tiny [1,U] carry fixups, ScalarE copies PSUM->SBUF,
    DMA streams x in and y out. Memory-bound by design.
"""

import os
import sys
import functools

sys.path.insert(0, "/opt/trn_rl_repo")
os.environ.setdefault("MYCRO_LOCAL_CACHE", "1")

import numpy as np

B, T, U = 16, 2048, 1024
NCORES = 8
BL = B // NCORES          # batches per core
L = 128                   # time block == partition dim
NBLK = T // L
H = 512                   # matmul moving-free max for fp32
NH = U // H
USE_F32R = os.environ.get("LOWPASS_F32R", "0") == "1"
GRP = int(os.environ.get("LOWPASS_GRP", "4"))


@functools.lru_cache(maxsize=8)
def _build(use_f32r: bool, grp: int = 4):
    """grp = time blocks per DMA group (grp*0.5MB per transfer).

    use_f32r: run the prefix matmul in float32r (full-rate fp32 path on
    the PE).  The x/A SBUF tiles are *declared* float32r so every
    producer (DMA, DVE fixup) emits that dtype, which the BIR verifier
    requires for FP32r matmul inputs.
    """
    import concourse.tile as tile
    from concourse import bacc, mybir

    nc = bacc.Bacc("TRN2", target_bir_lowering=False, debug=False)
    f32 = mybir.dt.float32
    mmdt = mybir.dt.float32r if use_f32r else f32
    x = nc.dram_tensor("x", [BL, T, U], f32, kind="ExternalInput").ap()
    at = nc.dram_tensor("at", [L, L], f32, kind="ExternalInput").ap()
    pc = nc.dram_tensor("pc", [1, L], f32, kind="ExternalInput").ap()
    c0 = nc.dram_tensor("c0", [1, U], f32, kind="ExternalInput").ap()
    y = nc.dram_tensor("y", [BL, T, U], f32, kind="ExternalOutput").ap()

    NG = NBLK // grp  # DMA groups per batch
    # dram view: [b, g, p, n, u] so one group DMA fills SBUF [128, grp*U]
    xr = x.rearrange("b (g n p) u -> b g p n u", n=grp, p=L)
    yr = y.rearrange("b (g n p) u -> b g p n u", n=grp, p=L)

    with tile.TileContext(nc) as tc:
        with (
            tc.tile_pool(name="const", bufs=1) as constp,
            tc.tile_pool(name="xin", bufs=3) as xinp,
            tc.tile_pool(name="yout", bufs=3) as youtp,
            tc.tile_pool(name="ypsum", bufs=8, space="PSUM") as ypp,
        ):
            att = constp.tile([L, L], mmdt)
            nc.sync.dma_start(att[:, :], at.bitcast(mmdt))
            pct = constp.tile([1, L], mmdt)
            nc.sync.dma_start(pct[:, :], pc.bitcast(mmdt))
            c0t = constp.tile([1, U], mmdt)
            nc.sync.dma_start(c0t[:, :], c0.bitcast(mmdt))

            # 4 independent carry chains: (batch, u-half).  Per block:
            #   yp = Arot @ x_blk           (mm1, PSUM start)
            #   yp += p_rot (x) carry       (mm2, PSUM accumulate)
            # where carry = previous block's output row 0 in SBUF (y rows
            # are rotated by +1, so the block's LAST time step sits at row
            # 0, which the ACT copy has already landed in the yt tile).
            # The chain lives entirely on PE + ACT; no DVE ops needed.
            prevc = [[c0t[0:1, h * H : (h + 1) * H] for h in range(NH)]
                     for _ in range(BL)]
            for g in range(NG):
                for b in range(BL):
                    xt = xinp.tile([L, grp * U], mmdt)
                    xt3 = xt[:, :].rearrange("p (n u) -> p n u", n=grp)
                    nc.sync.dma_start(xt3, xr[b, g].bitcast(mmdt))
                    yt = youtp.tile([L, grp * U], mmdt)
                    for n in range(grp):
                        for h in range(NH):
                            xb = xt[:, n * U + h * H : n * U + (h + 1) * H]
                            yp = ypp.tile([L, H], f32)
                            nc.tensor.matmul(
                                yp[:, :], lhsT=att[:, :], rhs=xb[:, :],
                                start=True, stop=False,
                            )
                            nc.tensor.matmul(
                                yp[:, :], lhsT=pct[:, :], rhs=prevc[b][h],
                                start=False, stop=True,
                            )
                            nc.scalar.activation(
                                yt[:, n * U + h * H : n * U + (h + 1) * H],
                                yp[:, :], mybir.ActivationFunctionType.Copy,
                            )
                            prevc[b][h] = yt[
                                0:1, n * U + h * H : n * U + (h + 1) * H
                            ]
                    # un-rotate: SBUF rows 1..127 -> DRAM rows 0..126,
                    # SBUF row 0 -> DRAM row 127 (two positive-stride DMAs)
                    yt3a = (yt[1:L, :].bitcast(f32)
                            .rearrange("p (n u) -> p n u", n=grp))
                    nc.gpsimd.dma_start(yr[b, g][0 : L - 1], yt3a)
                    yt3b = (yt[0:1, :].bitcast(f32)
                            .rearrange("p (n u) -> p n u", n=grp))
                    nc.gpsimd.dma_start(yr[b, g][L - 1 : L], yt3b)
    nc.compile()
    return nc


def _host_params(smoothing_var: np.ndarray, dtype=np.float64):
    """s (fp32 scalar, as reference computes it), A^T matrix, sf."""
    sm = smoothing_var.astype(np.float32).reshape(-1)
    s32 = (1.0 / (1.0 + np.exp(-sm.astype(np.float64)))).astype(np.float32)
    return s32


def _host_mats(s32_scalar):
    """Stationary matrix (row-reversed A, transposed for lhsT) and sf."""
    s = np.float64(s32_scalar)
    j = np.arange(L)[:, None]
    i = np.arange(L)[None, :]
    A = np.where(j >= i, (1.0 - s) * s ** (j - i), 0.0)
    Arot = np.roll(A, 1, axis=0)  # PSUM row m = y[(m-1) % 128]; row 0 = carry
    AT = np.ascontiguousarray(Arot.T.astype(np.float32))
    m = np.arange(L)
    pcol = (s ** (((m - 1) % L) + 1)).astype(np.float32).reshape(1, L)
    return AT, np.ascontiguousarray(pcol)


def kernel(inputs: np.ndarray, level_var: np.ndarray, smoothing_var: np.ndarray):
    from concourse import bass_utils

    x = np.ascontiguousarray(inputs, dtype=np.float32)
    assert x.shape == (B, T, U), x.shape
    s32 = _host_params(smoothing_var)
    if not np.all(s32 == s32[0]):
        # general per-unit s: fall back to exact numpy recurrence
        return _kernel_numpy(x, level_var, s32)
    AT, pcol = _host_mats(s32[0])
    c0 = np.ascontiguousarray(level_var.astype(np.float32).reshape(1, U))

    nc = _build(USE_F32R, GRP)
    in_maps = [
        {"x": np.ascontiguousarray(x[c * BL : (c + 1) * BL]), "at": AT,
         "pc": pcol, "c0": c0}
        for c in range(NCORES)
    ]
    res = bass_utils.run_bass_kernel_spmd(nc, in_maps, core_ids=list(range(NCORES)))
    out = np.concatenate([res.results[c]["y"] for c in range(NCORES)], axis=0)
    return out


def _kernel_numpy(x, level_var, s32):
    out = np.empty_like(x)
    c = np.broadcast_to(level_var.reshape(1, U), (x.shape[0], U)).astype(np.float32)
    one_minus = (1.0 - s32).astype(np.float32)
    for t in range(x.shape[1]):
        c = one_minus * x[:, t] + s32 * c
        out[:, t] = c
    return out


if __name__ == "__main__":
    rng = np.random.default_rng(0)
    xs = rng.standard_normal((B, T, U)).astype(np.float32)
    e = np.exp(-0.001 / 0.1)
    sm = np.full((1, U), np.log(e / (1 - e)), np.float32)
    lv = np.zeros((1, U), np.float32)
    o = kernel(xs, lv, sm)
    print("out", o.shape, o.dtype, float(np.abs(o).max()))
